# revision 8
# baseline (speedup 1.0000x reference)
"""GNN message-passing aggregator on 8 Trainium2 NeuronCores.

reference semantics:
    v[e]  = entity_emb[tail[e]] * weight[edge_type[e]]        # [E, D]
    out[n] = mean(v[e] for e with head[e] == n), 0 if none    # [N, D]

Strategy:
  * Host: sort edges by (head-block, tail-range, tail); shard contiguous
    64-entity blocks across the 8 cores -> each core owns a contiguous
    output slice, no cross-core reduction.
  * Edge chunks of 128 are single-(block, tail-range).  Rows are fetched
    with bulk `dma_gather` (int16 indices; the 100K-row table is split in
    4 ranges of 25000 rows, one gather op per range per group).
  * Per chunk, one DVE is_equal builds onehot[e, m] = (head[e]%64 == m);
    one PE matmul accumulates  acc[m, 0:65] += onehot^T @ [v | 1]  in PSUM
    over all chunks of a block (sums + counts in one pass).
  * Finalize per block: out = acc[:, :64] * 1/max(cnt, 1) on DVE+ACT.
"""

import sys

if "/opt/trn_rl_repo" not in sys.path:
    sys.path.insert(0, "/opt/trn_rl_repo")

import math

import numpy as np

import concourse.bacc as bacc
import concourse.bass as bass
import concourse.mybir as mybir
import concourse.tile as tile
from concourse.bass_utils import run_bass_kernel_spmd

F32 = mybir.dt.float32
I16 = mybir.dt.int16

N_CORES = 8
BLOCK = 64  # entities per output block (matmul stationary free dim)
D = 64
J = 64  # chunks (of 128 edges) per gather/compute group
NRANGE = 4  # tail-range splits of the entity table (int16 gather indices)


def _plan_and_stage(entity_emb, edge_index, edge_type, weight, n_cores=N_CORES, j_chunks=J):
    """Host-side preprocessing. Returns (plan dict, list of per-core in_maps)."""
    n_entities = entity_emb.shape[0]
    range_rows = -(-n_entities // NRANGE)  # rows per tail-range
    assert range_rows <= 32768
    head = np.ascontiguousarray(edge_index[0]).astype(np.int64)
    tail = np.ascontiguousarray(edge_index[1]).astype(np.int64)
    etype = np.asarray(edge_type).astype(np.int64)

    blocks_total = math.ceil(n_entities / BLOCK)
    bpc = math.ceil(blocks_total / n_cores)  # blocks per core
    core_rows = bpc * BLOCK

    blk_of = head // BLOCK
    rng_of = tail // range_rows
    order = np.lexsort((tail, rng_of, blk_of))
    head = head[order]
    tail = tail[order]
    etype = etype[order]
    blk_of = blk_of[order]
    rng_of = rng_of[order]

    core = blk_of // bpc
    lblk = blk_of % bpc
    seg = (core * bpc + lblk) * NRANGE + rng_of  # sorted, non-decreasing

    nseg = n_cores * bpc * NRANGE
    cnt = np.bincount(seg, minlength=nseg).reshape(n_cores, bpc * NRANGE)
    kbr = -(-cnt.max(axis=0) // 128)  # chunks per (lblk, range), may be 0
    # every block needs >= 1 chunk so its PSUM gets written
    kb_per_blk = kbr.reshape(bpc, NRANGE)
    empty = kb_per_blk.sum(axis=1) == 0
    kb_per_blk[empty, 0] = 1
    kbr = kb_per_blk.reshape(-1)

    total_chunks = int(kbr.sum())
    G = -(-total_chunks // j_chunks)
    kbr[-1] += G * j_chunks - total_chunks  # pad chunks on (last block, last range)
    chunk_base = np.zeros(nseg // n_cores + 1, np.int64)
    np.cumsum(kbr, out=chunk_base[1:])

    # natural chunk order: (lblk, range) -> final order: within each group of
    # j_chunks, stable-sort chunks by range so each range is one contiguous run
    nat_block = np.repeat(np.arange(bpc * NRANGE) // NRANGE, kbr)
    nat_rng = np.repeat(np.arange(bpc * NRANGE) % NRANGE, kbr)
    n_chunks = nat_block.shape[0]
    perm = np.empty(n_chunks, np.int64)  # natural idx -> final idx
    final_block = np.empty(n_chunks, np.int64)
    final_rng = np.empty(n_chunks, np.int64)
    for g in range(G):
        lo, hi = g * j_chunks, (g + 1) * j_chunks
        o = np.argsort(nat_rng[lo:hi], kind="stable")
        perm[lo + o] = np.arange(lo, hi)
        final_block[lo:hi] = nat_block[lo + o]
        final_rng[lo:hi] = nat_rng[lo + o]

    # gather runs per (group, range): (start_chunk_in_group, len_chunks)
    runs = []  # [G][NRANGE] -> (rs, rl)
    for g in range(G):
        row = []
        fr = final_rng[g * j_chunks : (g + 1) * j_chunks]
        for r in range(NRANGE):
            w = np.flatnonzero(fr == r)
            row.append((int(w[0]), int(w.size)) if w.size else (0, 0))
        runs.append(row)

    # per-block first/last chunk in FINAL order (for matmul start/stop)
    first_of_block = np.full(bpc, -1, np.int64)
    last_of_block = np.full(bpc, -1, np.int64)
    for ci in range(n_chunks):
        b = final_block[ci]
        if first_of_block[b] < 0:
            first_of_block[b] = ci
        last_of_block[b] = ci

    # edge placement: position within (core, lblk, range) segment
    _, idx_first, counts_u = np.unique(seg, return_index=True, return_counts=True)
    pos = np.arange(head.shape[0], dtype=np.int64) - np.repeat(idx_first, counts_u)
    nat_chunk = chunk_base[(lblk * NRANGE + rng_of)] + pos // 128
    fin_chunk = perm[nat_chunk]
    p = pos % 128
    g_idx = fin_chunk // j_chunks
    jj = fin_chunk % j_chunks
    flat = (g_idx * 128 + p) * j_chunks + jj  # slot in [G, 128, J] for we/hloc

    w65 = np.zeros((weight.shape[0], D + 1), np.float32)
    w65[:, :D] = np.asarray(weight, np.float32)
    w65[:, D] = 1.0

    emb_np = np.ascontiguousarray(np.asarray(entity_emb, np.float32))
    iota = np.broadcast_to(np.arange(BLOCK, dtype=np.float32), (128, BLOCK)).copy()

    # int16 gather index staging: [G, 128, J*128//16]; index i of group g
    # (i = chunk_in_group*128 + partition) lives at [g, 16k + i%16, i//16]
    idx_cols = j_chunks * 128 // 16
    in_maps = []
    nslots = G * 128 * j_chunks
    for c in range(n_cores):
        m = core == c
        fl = flat[m]
        hloc_a = np.full(nslots, -1.0, np.float32)
        hloc_a[fl] = (head[m] % BLOCK).astype(np.float32)
        we_a = np.zeros((nslots, D + 1), np.float32)
        we_a[fl] = w65[etype[m]]

        gpos = fin_chunk[m] * 128 + p[m]  # global gather position
        idx_flat = np.zeros(G * j_chunks * 128, np.int16)  # pads -> row 0 of range
        idx_flat[gpos] = (tail[m] % range_rows).astype(np.int16)
        idx_w = idx_flat.reshape(G, j_chunks * 128 // 16, 16)  # [G, i//16, i%16]
        idx_a = np.empty((G, 128, idx_cols), np.int16)
        for k in range(8):
            idx_a[:, 16 * k : 16 * (k + 1), :] = idx_w.transpose(0, 2, 1)

        in_maps.append(
            {
                "emb": emb_np,
                "idx16": idx_a,
                "hloc": hloc_a.reshape(G, 128, j_chunks),
                "we": we_a.reshape(G, 128, j_chunks, D + 1),
                "iota": iota,
            }
        )

    plan = dict(
        n_entities=n_entities,
        range_rows=range_rows,
        bpc=bpc,
        core_rows=core_rows,
        G=G,
        j_chunks=j_chunks,
        runs=runs,
        final_block=final_block,
        first_of_block=first_of_block,
        last_of_block=last_of_block,
        idx_cols=idx_cols,
    )
    return plan, in_maps


def _build_program(plan):
    n_entities = plan["n_entities"]
    range_rows = plan["range_rows"]
    bpc = plan["bpc"]
    G = plan["G"]
    JC = plan["j_chunks"]
    runs = plan["runs"]
    final_block = plan["final_block"]
    first_of_block = plan["first_of_block"]
    last_of_block = plan["last_of_block"]
    idx_cols = plan["idx_cols"]

    nc = bacc.Bacc()
    emb = nc.dram_tensor("emb", [n_entities, D], F32, kind="ExternalInput")
    idx16 = nc.dram_tensor("idx16", [G, 128, idx_cols], I16, kind="ExternalInput")
    hloc = nc.dram_tensor("hloc", [G, 128, JC], F32, kind="ExternalInput")
    we = nc.dram_tensor("we", [G, 128, JC, D + 1], F32, kind="ExternalInput")
    iota = nc.dram_tensor("iota", [128, BLOCK], F32, kind="ExternalInput")
    out = nc.dram_tensor("out", [bpc * BLOCK, D], F32, kind="ExternalOutput")

    with tile.TileContext(nc) as tc:
        with (
            tc.tile_pool(name="consts", bufs=1) as cpool,
            tc.tile_pool(name="io", bufs=2) as iopool,
            tc.tile_pool(name="oh", bufs=2) as ohpool,
            tc.tile_pool(name="fin", bufs=4) as finpool,
            tc.tile_pool(name="psum", bufs=4, space="PSUM") as psum_pool,
        ):
            iota_t = cpool.tile([128, BLOCK], F32)
            nc.sync.dma_start(out=iota_t[:], in_=iota[:])

            acc = {}
            for g in range(G):
                idx_t = iopool.tile([128, idx_cols], I16, tag="idx")
                nc.sync.dma_start(out=idx_t[:], in_=idx16[g])
                hloc_t = iopool.tile([128, JC], F32, tag="hloc")
                nc.sync.dma_start(out=hloc_t[:], in_=hloc[g])
                we_t = iopool.tile([128, JC, D + 1], F32, tag="we")
                nc.sync.dma_start(out=we_t[:], in_=we[g])

                g_t = iopool.tile([128, JC, D], F32, tag="gather")
                for r in range(NRANGE):
                    rs, rl = runs[g][r]
                    if rl == 0:
                        continue
                    lo = r * range_rows
                    hi = min(lo + range_rows, n_entities)
                    nc.gpsimd.dma_gather(
                        out_ap=g_t[:, rs : rs + rl, :],
                        in_ap=emb[lo:hi, :],
                        idxs_ap=idx_t[:, rs * 8 : (rs + rl) * 8],
                        num_idxs=rl * 128,
                        num_idxs_reg=rl * 128,
                        elem_size=D,
                        single_packet=False,
                    )

                # v = we[:, :, :64] * gathered   (in place in we_t)
                nc.vector.tensor_tensor(
                    out=we_t[:, :, :D],
                    in0=we_t[:, :, :D],
                    in1=g_t[:],
                    op=mybir.AluOpType.mult,
                )
                # onehot[p, j, m] = (hloc[p, j] == m)
                oh_t = ohpool.tile([128, JC, BLOCK], F32)
                nc.vector.tensor_tensor(
                    out=oh_t[:],
                    in0=hloc_t[:].unsqueeze(2).to_broadcast([128, JC, BLOCK]),
                    in1=iota_t[:].unsqueeze(1).to_broadcast([128, JC, BLOCK]),
                    op=mybir.AluOpType.is_equal,
                )

                for j in range(JC):
                    ci = g * JC + j
                    lb = int(final_block[ci])
                    first = ci == int(first_of_block[lb])
                    last = ci == int(last_of_block[lb])
                    if first:
                        acc[lb] = psum_pool.tile(
                            [BLOCK, D + 1], F32, name="acc", tag="acc"
                        )
                    nc.tensor.matmul(
                        out=acc[lb][:],
                        lhsT=oh_t[:, j, :],
                        rhs=we_t[:, j, :],
                        start=first,
                        stop=last,
                    )
                    if last:
                        a = acc.pop(lb)
                        cnt_t = finpool.tile([BLOCK, 1], F32, tag="cnt")
                        nc.vector.tensor_scalar_max(cnt_t[:], a[:, D : D + 1], 1.0)
                        rec_t = finpool.tile([BLOCK, 1], F32, tag="rec")
                        nc.vector.reciprocal(rec_t[:], cnt_t[:])
                        ob_t = finpool.tile([BLOCK, D], F32, tag="ob")
                        nc.scalar.activation(
                            out=ob_t[:],
                            in_=a[:, :D],
                            func=mybir.ActivationFunctionType.Copy,
                            scale=rec_t[:],
                        )
                        nc.sync.dma_start(
                            out=out[lb * BLOCK : (lb + 1) * BLOCK, :], in_=ob_t[:]
                        )
    return nc


def kernel(entity_emb, edge_index, edge_type, weight, _trace=False, _trace_kwargs=None):
    plan, in_maps = _plan_and_stage(entity_emb, edge_index, edge_type, weight)
    nc = _build_program(plan)
    nc.finalize()
    res = run_bass_kernel_spmd(
        nc,
        in_maps,
        core_ids=list(range(N_CORES)),
        trace=_trace,
        **(_trace_kwargs or {}),
    )
    outs = [res.results[c]["out"] for c in range(N_CORES)]
    full = np.concatenate(outs, axis=0)[: plan["n_entities"]]
    if _trace:
        kernel._last_results = res
    return full


# revision 10
# speedup vs baseline: 1.9287x; 1.9287x over previous
"""GNN message-passing aggregator on 8 Trainium2 NeuronCores.

reference semantics:
    v[e]  = entity_emb[tail[e]] * weight[edge_type[e]]        # [E, D]
    out[n] = mean(v[e] for e with head[e] == n), 0 if none    # [N, D]

Strategy:
  * Host: sort edges by (head-block, tail-range, tail); shard contiguous
    64-entity blocks across the 8 cores -> each core owns a contiguous
    output slice, no cross-core reduction.
  * Edge chunks of 128 are single-(block, tail-range).  Rows are fetched
    with bulk `dma_gather` (int16 indices; the 100K-row table is split in
    4 ranges of 25000 rows, one gather op per range per group).
  * Per chunk, one DVE is_equal builds onehot[e, m] = (head[e]%64 == m);
    one PE matmul accumulates  acc[m, 0:65] += onehot^T @ [v | 1]  in PSUM
    over all chunks of a block (sums + counts in one pass).
  * Finalize per block: out = acc[:, :64] * 1/max(cnt, 1) on DVE+ACT.
"""

import sys

if "/opt/trn_rl_repo" not in sys.path:
    sys.path.insert(0, "/opt/trn_rl_repo")

import math

import numpy as np

import concourse.bacc as bacc
import concourse.bass as bass
import concourse.mybir as mybir
import concourse.tile as tile
from concourse.bass_utils import run_bass_kernel_spmd

F32 = mybir.dt.float32
I16 = mybir.dt.int16

N_CORES = 8
BLOCK = 64  # entities per output block (matmul stationary free dim)
D = 64
J = 64  # chunks (of 128 edges) per gather/compute group
NRANGE = 4  # tail-range splits of the entity table (int16 gather indices)


def _plan_and_stage(entity_emb, edge_index, edge_type, weight, n_cores=N_CORES, j_chunks=J):
    """Host-side preprocessing. Returns (plan dict, list of per-core in_maps)."""
    n_entities = entity_emb.shape[0]
    range_rows = -(-n_entities // NRANGE)  # rows per tail-range
    assert range_rows <= 32768
    head = np.ascontiguousarray(edge_index[0]).astype(np.int64)
    tail = np.ascontiguousarray(edge_index[1]).astype(np.int64)
    etype = np.asarray(edge_type).astype(np.int64)

    blocks_total = math.ceil(n_entities / BLOCK)
    bpc = math.ceil(blocks_total / n_cores)  # blocks per core
    core_rows = bpc * BLOCK

    blk_of = head // BLOCK
    rng_of = tail // range_rows
    order = np.lexsort((tail, rng_of, blk_of))
    head = head[order]
    tail = tail[order]
    etype = etype[order]
    blk_of = blk_of[order]
    rng_of = rng_of[order]

    core = blk_of // bpc
    lblk = blk_of % bpc
    seg = (core * bpc + lblk) * NRANGE + rng_of  # sorted, non-decreasing

    nseg = n_cores * bpc * NRANGE
    cnt = np.bincount(seg, minlength=nseg).reshape(n_cores, bpc * NRANGE)
    kbr = -(-cnt.max(axis=0) // 128)  # chunks per (lblk, range), may be 0
    # every block needs >= 1 chunk so its PSUM gets written
    kb_per_blk = kbr.reshape(bpc, NRANGE)
    empty = kb_per_blk.sum(axis=1) == 0
    kb_per_blk[empty, 0] = 1
    kbr = kb_per_blk.reshape(-1)

    total_chunks = int(kbr.sum())
    G = -(-total_chunks // j_chunks)
    kbr[-1] += G * j_chunks - total_chunks  # pad chunks on (last block, last range)
    chunk_base = np.zeros(nseg // n_cores + 1, np.int64)
    np.cumsum(kbr, out=chunk_base[1:])

    # natural chunk order: (lblk, range) -> final order: within each group of
    # j_chunks, stable-sort chunks by range so each range is one contiguous run
    nat_block = np.repeat(np.arange(bpc * NRANGE) // NRANGE, kbr)
    nat_rng = np.repeat(np.arange(bpc * NRANGE) % NRANGE, kbr)
    n_chunks = nat_block.shape[0]
    perm = np.empty(n_chunks, np.int64)  # natural idx -> final idx
    final_block = np.empty(n_chunks, np.int64)
    final_rng = np.empty(n_chunks, np.int64)
    for g in range(G):
        lo, hi = g * j_chunks, (g + 1) * j_chunks
        o = np.argsort(nat_rng[lo:hi], kind="stable")
        perm[lo + o] = np.arange(lo, hi)
        final_block[lo:hi] = nat_block[lo + o]
        final_rng[lo:hi] = nat_rng[lo + o]

    # gather runs per (group, range): (start_chunk_in_group, len_chunks)
    runs = []  # [G][NRANGE] -> (rs, rl)
    for g in range(G):
        row = []
        fr = final_rng[g * j_chunks : (g + 1) * j_chunks]
        for r in range(NRANGE):
            w = np.flatnonzero(fr == r)
            row.append((int(w[0]), int(w.size)) if w.size else (0, 0))
        runs.append(row)

    # per-block first/last chunk in FINAL order (for matmul start/stop)
    first_of_block = np.full(bpc, -1, np.int64)
    last_of_block = np.full(bpc, -1, np.int64)
    for ci in range(n_chunks):
        b = final_block[ci]
        if first_of_block[b] < 0:
            first_of_block[b] = ci
        last_of_block[b] = ci

    # edge placement: position within (core, lblk, range) segment
    _, idx_first, counts_u = np.unique(seg, return_index=True, return_counts=True)
    pos = np.arange(head.shape[0], dtype=np.int64) - np.repeat(idx_first, counts_u)
    nat_chunk = chunk_base[(lblk * NRANGE + rng_of)] + pos // 128
    fin_chunk = perm[nat_chunk]
    p = pos % 128
    g_idx = fin_chunk // j_chunks
    jj = fin_chunk % j_chunks
    flat = (g_idx * 128 + p) * j_chunks + jj  # slot in [G, 128, J] for we/hloc

    w65 = np.zeros((weight.shape[0], D + 1), np.float32)
    w65[:, :D] = np.asarray(weight, np.float32)
    w65[:, D] = 1.0

    emb_np = np.ascontiguousarray(np.asarray(entity_emb, np.float32))
    iota = np.broadcast_to(np.arange(BLOCK, dtype=np.float32), (128, BLOCK)).copy()

    # int16 gather index staging: [G, 128, J*128//16]; index i of group g
    # (i = chunk_in_group*128 + partition) lives at [g, 16k + i%16, i//16]
    idx_cols = j_chunks * 128 // 16
    in_maps = []
    nslots = G * 128 * j_chunks
    for c in range(n_cores):
        m = core == c
        fl = flat[m]
        hloc_a = np.full(nslots, -1.0, np.float32)
        hloc_a[fl] = (head[m] % BLOCK).astype(np.float32)
        we_a = np.zeros((nslots, D + 1), np.float32)
        we_a[fl] = w65[etype[m]]

        gpos = fin_chunk[m] * 128 + p[m]  # global gather position
        idx_flat = np.zeros(G * j_chunks * 128, np.int16)  # pads -> row 0 of range
        idx_flat[gpos] = (tail[m] % range_rows).astype(np.int16)
        idx_w = idx_flat.reshape(G, j_chunks * 128 // 16, 16)  # [G, i//16, i%16]
        idx_a = np.empty((G, 128, idx_cols), np.int16)
        for k in range(8):
            idx_a[:, 16 * k : 16 * (k + 1), :] = idx_w.transpose(0, 2, 1)

        in_maps.append(
            {
                "emb": emb_np,
                "idx16": idx_a,
                "hloc": hloc_a.reshape(G, 128, j_chunks),
                "we": we_a.reshape(G, 128, j_chunks, D + 1),
                "iota": iota,
            }
        )

    plan = dict(
        n_entities=n_entities,
        range_rows=range_rows,
        bpc=bpc,
        core_rows=core_rows,
        G=G,
        j_chunks=j_chunks,
        runs=runs,
        final_block=final_block,
        first_of_block=first_of_block,
        last_of_block=last_of_block,
        idx_cols=idx_cols,
    )
    return plan, in_maps


def _build_program(plan):
    n_entities = plan["n_entities"]
    range_rows = plan["range_rows"]
    bpc = plan["bpc"]
    G = plan["G"]
    JC = plan["j_chunks"]
    runs = plan["runs"]
    final_block = plan["final_block"]
    first_of_block = plan["first_of_block"]
    last_of_block = plan["last_of_block"]
    idx_cols = plan["idx_cols"]

    nc = bacc.Bacc(num_swdge_queues=4)
    emb = nc.dram_tensor("emb", [n_entities, D], F32, kind="ExternalInput")
    idx16 = nc.dram_tensor("idx16", [G, 128, idx_cols], I16, kind="ExternalInput")
    hloc = nc.dram_tensor("hloc", [G, 128, JC], F32, kind="ExternalInput")
    we = nc.dram_tensor("we", [G, 128, JC, D + 1], F32, kind="ExternalInput")
    iota = nc.dram_tensor("iota", [128, BLOCK], F32, kind="ExternalInput")
    out = nc.dram_tensor("out", [bpc * BLOCK, D], F32, kind="ExternalOutput")

    with tile.TileContext(nc) as tc:
        with (
            tc.tile_pool(name="consts", bufs=1) as cpool,
            tc.tile_pool(name="io", bufs=2) as iopool,
            tc.tile_pool(name="oh", bufs=2) as ohpool,
            tc.tile_pool(name="fin", bufs=4) as finpool,
            tc.tile_pool(name="psum", bufs=4, space="PSUM") as psum_pool,
        ):
            iota_t = cpool.tile([128, BLOCK], F32)
            nc.sync.dma_start(out=iota_t[:], in_=iota[:])

            acc = {}
            for g in range(G):
                idx_t = iopool.tile([128, idx_cols], I16, tag="idx")
                nc.sync.dma_start(out=idx_t[:], in_=idx16[g])
                hloc_t = iopool.tile([128, JC], F32, tag="hloc")
                nc.sync.dma_start(out=hloc_t[:], in_=hloc[g])
                we_t = iopool.tile([128, JC, D + 1], F32, tag="we")
                nc.sync.dma_start(out=we_t[:], in_=we[g])

                g_t = iopool.tile([128, JC, D], F32, tag="gather")
                for r in range(NRANGE):
                    rs, rl = runs[g][r]
                    if rl == 0:
                        continue
                    lo = r * range_rows
                    hi = min(lo + range_rows, n_entities)
                    nc.gpsimd.dma_gather(
                        out_ap=g_t[:, rs : rs + rl, :],
                        in_ap=emb[lo:hi, :],
                        idxs_ap=idx_t[:, rs * 8 : (rs + rl) * 8],
                        num_idxs=rl * 128,
                        num_idxs_reg=rl * 128,
                        elem_size=D,
                        single_packet=False,
                        queue_num=r,
                    )

                # v = we[:, :, :64] * gathered   (in place in we_t)
                nc.vector.tensor_tensor(
                    out=we_t[:, :, :D],
                    in0=we_t[:, :, :D],
                    in1=g_t[:],
                    op=mybir.AluOpType.mult,
                )
                # onehot[p, j, m] = (hloc[p, j] == m)
                oh_t = ohpool.tile([128, JC, BLOCK], F32)
                nc.vector.tensor_tensor(
                    out=oh_t[:],
                    in0=hloc_t[:].unsqueeze(2).to_broadcast([128, JC, BLOCK]),
                    in1=iota_t[:].unsqueeze(1).to_broadcast([128, JC, BLOCK]),
                    op=mybir.AluOpType.is_equal,
                )

                for j in range(JC):
                    ci = g * JC + j
                    lb = int(final_block[ci])
                    first = ci == int(first_of_block[lb])
                    last = ci == int(last_of_block[lb])
                    if first:
                        acc[lb] = psum_pool.tile(
                            [BLOCK, D + 1], F32, name="acc", tag="acc"
                        )
                    nc.tensor.matmul(
                        out=acc[lb][:],
                        lhsT=oh_t[:, j, :],
                        rhs=we_t[:, j, :],
                        start=first,
                        stop=last,
                    )
                    if last:
                        a = acc.pop(lb)
                        cnt_t = finpool.tile([BLOCK, 1], F32, tag="cnt")
                        nc.vector.tensor_scalar_max(cnt_t[:], a[:, D : D + 1], 1.0)
                        rec_t = finpool.tile([BLOCK, 1], F32, tag="rec")
                        nc.vector.reciprocal(rec_t[:], cnt_t[:])
                        ob_t = finpool.tile([BLOCK, D], F32, tag="ob")
                        nc.scalar.activation(
                            out=ob_t[:],
                            in_=a[:, :D],
                            func=mybir.ActivationFunctionType.Copy,
                            scale=rec_t[:],
                        )
                        nc.sync.dma_start(
                            out=out[lb * BLOCK : (lb + 1) * BLOCK, :], in_=ob_t[:]
                        )
    return nc


def kernel(entity_emb, edge_index, edge_type, weight, _trace=False, _trace_kwargs=None):
    plan, in_maps = _plan_and_stage(entity_emb, edge_index, edge_type, weight)
    nc = _build_program(plan)
    nc.finalize()
    res = run_bass_kernel_spmd(
        nc,
        in_maps,
        core_ids=list(range(N_CORES)),
        trace=_trace,
        **(_trace_kwargs or {}),
    )
    outs = [res.results[c]["out"] for c in range(N_CORES)]
    full = np.concatenate(outs, axis=0)[: plan["n_entities"]]
    if _trace:
        kernel._last_results = res
    return full


# revision 11
# speedup vs baseline: 2.7824x; 1.4426x over previous
"""GNN message-passing aggregator on 8 Trainium2 NeuronCores.

reference semantics:
    v[e]  = entity_emb[tail[e]] * weight[edge_type[e]]        # [E, D]
    out[n] = mean(v[e] for e with head[e] == n), 0 if none    # [N, D]

Strategy:
  * Host: sort edges by (head-block, tail-range, tail); shard contiguous
    64-entity blocks across the 8 cores -> each core owns a contiguous
    output slice, no cross-core reduction.
  * Edge chunks of 128 are single-(block, tail-range).  Rows are fetched
    with bulk `dma_gather` (int16 indices; the 100K-row table is split in
    4 ranges of 25000 rows, one gather op per range per group).
  * Per chunk, one DVE is_equal builds onehot[e, m] = (head[e]%64 == m);
    one PE matmul accumulates  acc[m, 0:65] += onehot^T @ [v | 1]  in PSUM
    over all chunks of a block (sums + counts in one pass).
  * Finalize per block: out = acc[:, :64] * 1/max(cnt, 1) on DVE+ACT.
"""

import sys

if "/opt/trn_rl_repo" not in sys.path:
    sys.path.insert(0, "/opt/trn_rl_repo")

import math

import numpy as np

import concourse.bacc as bacc
import concourse.bass as bass
import concourse.mybir as mybir
import concourse.tile as tile
from concourse.bass_utils import run_bass_kernel_spmd

F32 = mybir.dt.float32
I16 = mybir.dt.int16

N_CORES = 8
BLOCK = 64  # entities per output block (matmul stationary free dim)
D = 64
J = 64  # chunks (of 128 edges) per gather/compute group
NRANGE = 4  # tail-range splits of the entity table (int16 gather indices)


def _plan_and_stage(entity_emb, edge_index, edge_type, weight, n_cores=N_CORES, j_chunks=J):
    """Host-side preprocessing. Returns (plan dict, list of per-core in_maps)."""
    n_entities = entity_emb.shape[0]
    range_rows = -(-n_entities // NRANGE)  # rows per tail-range
    assert range_rows <= 32768
    head = np.ascontiguousarray(edge_index[0]).astype(np.int64)
    tail = np.ascontiguousarray(edge_index[1]).astype(np.int64)
    etype = np.asarray(edge_type).astype(np.int64)

    blocks_total = math.ceil(n_entities / BLOCK)
    bpc = math.ceil(blocks_total / n_cores)  # blocks per core
    core_rows = bpc * BLOCK

    blk_of = head // BLOCK
    rng_of = tail // range_rows
    order = np.lexsort((tail, rng_of, blk_of))
    head = head[order]
    tail = tail[order]
    etype = etype[order]
    blk_of = blk_of[order]
    rng_of = rng_of[order]

    core = blk_of // bpc
    lblk = blk_of % bpc
    seg = (core * bpc + lblk) * NRANGE + rng_of  # sorted, non-decreasing

    nseg = n_cores * bpc * NRANGE
    cnt = np.bincount(seg, minlength=nseg).reshape(n_cores, bpc * NRANGE)
    kbr = -(-cnt.max(axis=0) // 128)  # chunks per (lblk, range), may be 0
    # every block needs >= 1 chunk so its PSUM gets written
    kb_per_blk = kbr.reshape(bpc, NRANGE)
    empty = kb_per_blk.sum(axis=1) == 0
    kb_per_blk[empty, 0] = 1
    kbr = kb_per_blk.reshape(-1)

    total_chunks = int(kbr.sum())
    G = -(-total_chunks // j_chunks)
    kbr[-1] += G * j_chunks - total_chunks  # pad chunks on (last block, last range)
    chunk_base = np.zeros(nseg // n_cores + 1, np.int64)
    np.cumsum(kbr, out=chunk_base[1:])

    # natural chunk order: (lblk, range) -> final order: within each group of
    # j_chunks, stable-sort chunks by range so each range is one contiguous run
    nat_block = np.repeat(np.arange(bpc * NRANGE) // NRANGE, kbr)
    nat_rng = np.repeat(np.arange(bpc * NRANGE) % NRANGE, kbr)
    n_chunks = nat_block.shape[0]
    perm = np.empty(n_chunks, np.int64)  # natural idx -> final idx
    final_block = np.empty(n_chunks, np.int64)
    final_rng = np.empty(n_chunks, np.int64)
    for g in range(G):
        lo, hi = g * j_chunks, (g + 1) * j_chunks
        o = np.argsort(nat_rng[lo:hi], kind="stable")
        perm[lo + o] = np.arange(lo, hi)
        final_block[lo:hi] = nat_block[lo + o]
        final_rng[lo:hi] = nat_rng[lo + o]

    # gather runs per (group, range): (start_chunk_in_group, len_chunks)
    runs = []  # [G][NRANGE] -> (rs, rl)
    for g in range(G):
        row = []
        fr = final_rng[g * j_chunks : (g + 1) * j_chunks]
        for r in range(NRANGE):
            w = np.flatnonzero(fr == r)
            row.append((int(w[0]), int(w.size)) if w.size else (0, 0))
        runs.append(row)

    # per-block first/last chunk in FINAL order (for matmul start/stop)
    first_of_block = np.full(bpc, -1, np.int64)
    last_of_block = np.full(bpc, -1, np.int64)
    for ci in range(n_chunks):
        b = final_block[ci]
        if first_of_block[b] < 0:
            first_of_block[b] = ci
        last_of_block[b] = ci

    # edge placement: position within (core, lblk, range) segment
    _, idx_first, counts_u = np.unique(seg, return_index=True, return_counts=True)
    pos = np.arange(head.shape[0], dtype=np.int64) - np.repeat(idx_first, counts_u)
    nat_chunk = chunk_base[(lblk * NRANGE + rng_of)] + pos // 128
    fin_chunk = perm[nat_chunk]
    p = pos % 128
    g_idx = fin_chunk // j_chunks
    jj = fin_chunk % j_chunks
    flat = (g_idx * 128 + p) * j_chunks + jj  # slot in [G, 128, J] for we/hloc

    w65 = np.zeros((weight.shape[0], D + 1), np.float32)
    w65[:, :D] = np.asarray(weight, np.float32)
    w65[:, D] = 1.0

    emb_np = np.ascontiguousarray(np.asarray(entity_emb, np.float32))
    iota = np.broadcast_to(np.arange(BLOCK, dtype=np.float32), (128, BLOCK)).copy()

    # int16 gather index staging: [G, 128, J*128//16]; index i of group g
    # (i = chunk_in_group*128 + partition) lives at [g, 16k + i%16, i//16]
    idx_cols = j_chunks * 128 // 16
    in_maps = []
    nslots = G * 128 * j_chunks
    for c in range(n_cores):
        m = core == c
        fl = flat[m]
        hloc_a = np.full(nslots, -1.0, np.float32)
        hloc_a[fl] = (head[m] % BLOCK).astype(np.float32)
        we_a = np.zeros((nslots, D + 1), np.float32)
        we_a[fl] = w65[etype[m]]

        gpos = fin_chunk[m] * 128 + p[m]  # global gather position
        idx_flat = np.zeros(G * j_chunks * 128, np.int16)  # pads -> row 0 of range
        idx_flat[gpos] = (tail[m] % range_rows).astype(np.int16)
        idx_w = idx_flat.reshape(G, j_chunks * 128 // 16, 16)  # [G, i//16, i%16]
        idx_a = np.empty((G, 128, idx_cols), np.int16)
        for k in range(8):
            idx_a[:, 16 * k : 16 * (k + 1), :] = idx_w.transpose(0, 2, 1)

        in_maps.append(
            {
                "emb": emb_np,
                "idx16": idx_a,
                "hloc": hloc_a.reshape(G, 128, j_chunks),
                "we": we_a.reshape(G, 128, j_chunks, D + 1),
                "iota": iota,
            }
        )

    plan = dict(
        n_entities=n_entities,
        range_rows=range_rows,
        bpc=bpc,
        core_rows=core_rows,
        G=G,
        j_chunks=j_chunks,
        runs=runs,
        final_block=final_block,
        first_of_block=first_of_block,
        last_of_block=last_of_block,
        idx_cols=idx_cols,
    )
    return plan, in_maps


def _build_program(plan):
    n_entities = plan["n_entities"]
    range_rows = plan["range_rows"]
    bpc = plan["bpc"]
    G = plan["G"]
    JC = plan["j_chunks"]
    runs = plan["runs"]
    final_block = plan["final_block"]
    first_of_block = plan["first_of_block"]
    last_of_block = plan["last_of_block"]
    idx_cols = plan["idx_cols"]

    nc = bacc.Bacc(num_swdge_queues=4)
    emb = nc.dram_tensor("emb", [n_entities, D], F32, kind="ExternalInput")
    idx16 = nc.dram_tensor("idx16", [G, 128, idx_cols], I16, kind="ExternalInput")
    hloc = nc.dram_tensor("hloc", [G, 128, JC], F32, kind="ExternalInput")
    we = nc.dram_tensor("we", [G, 128, JC, D + 1], F32, kind="ExternalInput")
    iota = nc.dram_tensor("iota", [128, BLOCK], F32, kind="ExternalInput")
    out = nc.dram_tensor("out", [bpc * BLOCK, D], F32, kind="ExternalOutput")

    with tile.TileContext(nc) as tc:
        with (
            tc.tile_pool(name="consts", bufs=1) as cpool,
            tc.tile_pool(name="io", bufs=3) as iopool,
            tc.tile_pool(name="oh", bufs=2) as ohpool,
            tc.tile_pool(name="fin", bufs=4) as finpool,
            tc.tile_pool(name="psum", bufs=6, space="PSUM") as psum_pool,
        ):
            iota_t = cpool.tile([128, BLOCK], F32)
            nc.sync.dma_start(out=iota_t[:], in_=iota[:])

            acc = {}
            for g in range(G):
                idx_t = iopool.tile([128, idx_cols], I16, tag="idx")
                nc.sync.dma_start(out=idx_t[:], in_=idx16[g])
                hloc_t = iopool.tile([128, JC], F32, tag="hloc")
                nc.sync.dma_start(out=hloc_t[:], in_=hloc[g])
                we_t = iopool.tile([128, JC, D + 1], F32, tag="we")
                nc.sync.dma_start(out=we_t[:], in_=we[g])

                g_t = iopool.tile([128, JC, D], F32, tag="gather")
                for r in range(NRANGE):
                    rs, rl = runs[g][r]
                    if rl == 0:
                        continue
                    lo = r * range_rows
                    hi = min(lo + range_rows, n_entities)
                    nc.gpsimd.dma_gather(
                        out_ap=g_t[:, rs : rs + rl, :],
                        in_ap=emb[lo:hi, :],
                        idxs_ap=idx_t[:, rs * 8 : (rs + rl) * 8],
                        num_idxs=rl * 128,
                        num_idxs_reg=rl * 128,
                        elem_size=D,
                        single_packet=False,
                        queue_num=r,
                    )

                # v = we[:, :, :64] * gathered   (in place in we_t)
                nc.vector.tensor_tensor(
                    out=we_t[:, :, :D],
                    in0=we_t[:, :, :D],
                    in1=g_t[:],
                    op=mybir.AluOpType.mult,
                )
                # onehot[p, j, m] = (hloc[p, j] == m)
                oh_t = ohpool.tile([128, JC, BLOCK], F32)
                nc.vector.tensor_tensor(
                    out=oh_t[:],
                    in0=hloc_t[:].unsqueeze(2).to_broadcast([128, JC, BLOCK]),
                    in1=iota_t[:].unsqueeze(1).to_broadcast([128, JC, BLOCK]),
                    op=mybir.AluOpType.is_equal,
                )

                for j in range(JC):
                    ci = g * JC + j
                    lb = int(final_block[ci])
                    first = ci == int(first_of_block[lb])
                    last = ci == int(last_of_block[lb])
                    if first:
                        acc[lb] = psum_pool.tile(
                            [BLOCK, D + 1], F32, name="acc", tag="acc"
                        )
                    nc.tensor.matmul(
                        out=acc[lb][:],
                        lhsT=oh_t[:, j, :],
                        rhs=we_t[:, j, :],
                        start=first,
                        stop=last,
                    )
                    if last:
                        a = acc.pop(lb)
                        cnt_t = finpool.tile([BLOCK, 1], F32, tag="cnt")
                        nc.vector.tensor_scalar_max(cnt_t[:], a[:, D : D + 1], 1.0)
                        rec_t = finpool.tile([BLOCK, 1], F32, tag="rec")
                        nc.vector.reciprocal(rec_t[:], cnt_t[:])
                        ob_t = finpool.tile([BLOCK, D], F32, tag="ob")
                        nc.scalar.activation(
                            out=ob_t[:],
                            in_=a[:, :D],
                            func=mybir.ActivationFunctionType.Copy,
                            scale=rec_t[:],
                        )
                        nc.sync.dma_start(
                            out=out[lb * BLOCK : (lb + 1) * BLOCK, :], in_=ob_t[:]
                        )
    return nc


def kernel(entity_emb, edge_index, edge_type, weight, _trace=False, _trace_kwargs=None):
    plan, in_maps = _plan_and_stage(entity_emb, edge_index, edge_type, weight)
    nc = _build_program(plan)
    nc.finalize()
    res = run_bass_kernel_spmd(
        nc,
        in_maps,
        core_ids=list(range(N_CORES)),
        trace=_trace,
        **(_trace_kwargs or {}),
    )
    outs = [res.results[c]["out"] for c in range(N_CORES)]
    full = np.concatenate(outs, axis=0)[: plan["n_entities"]]
    if _trace:
        kernel._last_results = res
    return full
